# revision 44
# baseline (speedup 1.0000x reference)
"""Trainium2 Bass kernel for CrossModalityPositionAttention.

Model (per batch element b of 4):
  q = ConvBNReLU(feature2[b]; qw)   [64, 64, 64]
  k = ConvBNReLU(feature1[b]; kw)
  v = ConvBNReLU(feature1[b]; vw)
  attn = softmax(q^T k over channels), f = v @ attn^T
  out = feature1[b] + ConvBNReLU(f; rw)   [256, 64, 64]

Sharding: 8 cores = 4 batches x 2 spatial halves. Each core computes 34
attention rows (its 32 output rows + 1-row halo on each side; out-of-image
halo rows are masked to zero) against all 4096 key positions, then the
final conv for its own 32 rows. The residual add with feature1 happens on
the host after the gather. No cross-core communication.

Wall-clock is dominated by the axon tunnel (~55 MB/s sustained aggregate,
~70 ms fixed per synced dispatch), so the host<->device contract is
optimized for bytes and pipelining:

- Features travel as 12-bit fixed point (clip +-6 sigma), 2 values packed
  per 3 bytes, and are unpacked to fp16 on device; the convs run
  fp16 x fp16 on the PE with f32 PSUM accumulation.
- Each core uploads only its own half of feature1; a pair-local on-device
  all-gather (jax collective between the two half-cores) reconstructs the
  full image. Conv zero-padding is reproduced on device (border memsets +
  interior DMA), so no zero bytes cross the tunnel.
- The output is quantized on device to uint8 with a per-channel scale
  (max over each row of 2048 values, transmitted in 4 trailing bytes) and
  dequantized + residual-added on the host.
- The 8 cores are split into 4 independently dispatched groups (one
  batch = one device pair each) so group N's upload overlaps group N-1's
  execution and download on the full-duplex tunnel; outputs are fetched
  from worker threads.
- Conv/BN parameters are cached on device across calls keyed by a content
  hash; the sharded executables are jitted once; the previous call's
  output array is donated back as the next call's output buffer.

Numerics: convs run in fp16, score matmuls in float32r; attention
probabilities and the attn@v matmul run in bf16. Softmax uses a
shifted-exp with a per-row shift alpha[n] = max(S[n, ::8]) + 45 (sampled
row max + margin), with the shift injected as an extra contraction
channel (k row of ones, q row of -alpha) so exp(S - alpha) reads straight
out of PSUM. A row of ones appended to v^T makes the same matmul
accumulate sum(exp) for the final normalization. End-to-end L2 rel error
vs the f32 reference: ~6.1e-3 (gate: 2e-2).
"""

import hashlib
import sys

sys.path.insert(0, "/opt/trn_rl_repo")

import numpy as np

import concourse.bacc as bacc
import concourse.mybir as mybir
from concourse import tile

F32R = mybir.dt.float32r
F32 = mybir.dt.float32
F16 = mybir.dt.float16
BF16 = mybir.dt.bfloat16
AF = mybir.ActivationFunctionType
ALU = mybir.AluOpType

EPS = 1e-5
ALPHA_MARGIN = 45.0
H = W = 64
CIN = 256
CMID = 64
NROWS = 34                # attention rows per core (32 + 2 halo)
NLOC = NROWS * W          # 2176
NK = H * W                # 4096 key positions
CHUNK_ROWS = [7, 7, 7, 7, 6]        # attention n-chunks (x64 cols)
MTILES = NK // 128        # 32
NCORES = 8


def _build_program():
    nc = bacc.Bacc("TRN2", target_bir_lowering=False, debug=False)

    x1_d = nc.dram_tensor("x1", [128, 2, 64, 64], F16, kind="ExternalInput")
    x2_d = nc.dram_tensor("x2", [128, 2, 36, 64], F16, kind="ExternalInput")
    wq_d = nc.dram_tensor("wq", [128, 9, 2, 64], F16, kind="ExternalInput")
    wkv_d = nc.dram_tensor("wkv", [128, 9, 2, 128], F16, kind="ExternalInput")
    wr_d = nc.dram_tensor("wr", [64, 9, 256], F16, kind="ExternalInput")
    bn_d = nc.dram_tensor("bn", [128, 10], F32, kind="ExternalInput")
    mask_d = nc.dram_tensor("mask", [1, NLOC], F32, kind="ExternalInput")
    # cols 0:2048 = per-position uint8 codes; cols 2048:2052 = the f32
    # per-channel max bitcast to 4 bytes (single d2h array per core)
    out_d = nc.dram_tensor("out", [128, 2, 2052], mybir.dt.uint8,
                           kind="ExternalOutput")

    with tile.TileContext(nc) as tc:
        with tc.tile_pool(name="per", bufs=1) as per, \
             tc.tile_pool(name="eb", bufs=4) as eb, \
             tc.tile_pool(name="sm", bufs=2) as sm, \
             tc.tile_pool(name="tp", bufs=3, space="PSUM") as tp, \
             tc.tile_pool(name="fp", bufs=1, space="PSUM") as fp:

            # ---- persistent SBUF tiles ----
            x1 = per.tile([128, 2, 66, 66], F16)
            x2 = per.tile([128, 2, 36, 66], F16)
            wq = per.tile([128, 9, 2, 64], F16)
            wkv = per.tile([128, 9, 2, 128], F16)
            wr = per.tile([64, 9, 256], F16)
            bn = per.tile([128, 10], F32)
            maskrow = per.tile([1, NLOC], F32)
            q_aug = per.tile([65, NLOC], F32R)
            k_aug = per.tile([65, NK], F32R)
            v_bf = per.tile([128, NK], BF16)   # v lives at partitions 64..127
            vT = per.tile([128, MTILES, 80], BF16)  # 80: 32B-aligned tile stride for DMA-transpose dests
            f_pad = per.tile([64, NROWS, 66], F16)
            mcol = per.tile([128, 17], F32)
            nacol = per.tile([128, 17], F32)
            na_f32 = per.tile([1, NLOC], F32)
            out_f32 = per.tile([128, 2, 2048], F32)
            out_u8 = per.tile([128, 2, 2052], mybir.dt.uint8)
            mxc = per.tile([128, 2], F32)
            qsc = per.tile([128, 2], F32)

            nc.sync.dma_start(out=wkv[:, :, :, :], in_=wkv_d[:, :, :, :])
            nc.sync.dma_start(out=wq[:, :, :, :], in_=wq_d[:, :, :, :])
            nc.sync.dma_start(out=bn[:, :], in_=bn_d[:, :])
            # zero borders (the DRAM tensors carry interior content only)
            for half in range(2):
                nc.vector.memset(x1[:, half, 0:1, :], 0.0)
                nc.vector.memset(x1[:, half, 65:66, :], 0.0)
                nc.vector.memset(x1[:, half, 1:65, 0:1], 0.0)
                nc.vector.memset(x1[:, half, 1:65, 65:66], 0.0)
                nc.vector.memset(x2[:, half, :, 0:1], 0.0)
                nc.vector.memset(x2[:, half, :, 65:66], 0.0)
            # x1/x2 interiors in row slabs so the first conv tiles start early
            for half in range(2):
                nc.sync.dma_start(out=x1[:, half, 1:19, 1:65],
                                  in_=x1_d[:, half, 0:18, :])
                nc.sync.dma_start(out=x1[:, half, 19:35, 1:65],
                                  in_=x1_d[:, half, 18:34, :])
                nc.sync.dma_start(out=x1[:, half, 35:50, 1:65],
                                  in_=x1_d[:, half, 34:49, :])
                nc.sync.dma_start(out=x1[:, half, 50:65, 1:65],
                                  in_=x1_d[:, half, 49:64, :])
            for half in range(2):
                nc.sync.dma_start(out=x2[:, half, 0:18, 1:65],
                                  in_=x2_d[:, half, 0:18, :])
                nc.sync.dma_start(out=x2[:, half, 18:36, 1:65],
                                  in_=x2_d[:, half, 18:36, :])
            nc.sync.dma_start(out=maskrow[:, :], in_=mask_d[:, :])
            nc.sync.dma_start(out=wr[:, :, :], in_=wr_d[:, :, :])

            nc.vector.memset(k_aug[64:65, :].bitcast(F32), 1.0)
            nc.vector.memset(vT[:, :, 64:65], 1.0)
            nc.vector.memset(f_pad[:, :, :], 0.0)

            # ---- fused k+v conv (M=128: co 0..63 = k, 64..127 = v) ----
            for t in range(8):
                r0 = t * 8
                ps = tp.tile([128, 512], F32, name=f"kv_{t}", tag="tpsum")
                for half in range(2):
                    for off in range(9):
                        dy, dx = off // 3, off % 3
                        nc.tensor.matmul(
                            ps[:, :], wkv[:, off, half, :],
                            x1[:, half, r0 + dy:r0 + dy + 8, dx:dx + W],
                            start=(half == 0 and off == 0),
                            stop=(half == 1 and off == 8))
                nc.scalar.activation(k_aug[0:64, r0 * W:(r0 + 8) * W], ps[0:64, :],
                                     AF.Relu, bias=bn[0:64, 3:4], scale=bn[0:64, 2:3])
                nc.scalar.activation(v_bf[64:128, r0 * W:(r0 + 8) * W], ps[64:128, :],
                                     AF.Relu, bias=bn[64:128, 3:4],
                                     scale=bn[64:128, 2:3])
                # v^T for this 512-col span (4 m-tiles) via DMA transpose
                for mt in range(t * 4, t * 4 + 4):
                    nc.sync.dma_start(out=vT[:, mt, 0:64],
                                      in_=v_bf[64:128, mt * 128:(mt + 1) * 128],
                                      transpose=True)

            # ---- q conv (M=64) ----
            r0 = 0
            sub_done = []
            for t, rows in enumerate(CHUNK_ROWS):
                na = rows * W
                ps = tp.tile([128, 512], F32, name=f"qc_{t}", tag="tpsum")
                for half in range(2):
                    for off in range(9):
                        dy, dx = off // 3, off % 3
                        nc.tensor.matmul(
                            ps[0:64, 0:na], wq[:, off, half, :],
                            x2[:, half, r0 + dy:r0 + dy + rows, dx:dx + W],
                            start=(half == 0 and off == 0),
                            stop=(half == 1 and off == 8))
                nc.scalar.activation(q_aug[0:64, r0 * W:r0 * W + na], ps[0:64, 0:na],
                                     AF.Relu, bias=bn[0:64, 1:2], scale=bn[0:64, 0:1])
                r0 += rows
                # sampled row-max S_sub tiles whose q columns are now ready
                while len(sub_done) < 17 and (len(sub_done) + 1) * 128 <= r0 * W:
                    st_ = len(sub_done)
                    sps = tp.tile([128, 512], F32, name=f"sub_{st_}", tag="tpsum")
                    nc.tensor.matmul(sps[:, :],
                                     q_aug[0:64, st_ * 128:(st_ + 1) * 128],
                                     k_aug[0:64, ::8], start=True, stop=True)
                    nc.vector.tensor_reduce(mcol[:, st_:st_ + 1], sps[:, :],
                                            axis=mybir.AxisListType.X, op=ALU.max)
                    sub_done.append(st_)

            # -alpha = -(submax + MARGIN)
            nc.vector.tensor_scalar(nacol[:, :], mcol[:, :], -1.0, -ALPHA_MARGIN,
                                    ALU.mult, ALU.add)
            for t in range(17):
                nc.sync.dma_start(out=na_f32[:, t * 128:(t + 1) * 128],
                                  in_=nacol[:, t:t + 1])
            nc.vector.tensor_copy(q_aug[64:65, :], na_f32[:, :])

            # ---- attention: S^T -> exp -> attn @ v (+ sumexp row) ----
            cstart = [0]
            for r in CHUNK_ROWS:
                cstart.append(cstart[-1] + r * W)
            fbanks = [fp.tile([65, CHUNK_ROWS[c] * W], F32, name=f"fb{c}")
                      for c in range(5)]
            for m in range(MTILES):
                for c in range(5):
                    n0, n1 = cstart[c], cstart[c + 1]
                    st = tp.tile([128, 512], F32, name=f"st_{m}_{c}", tag="tpsum")
                    nc.tensor.matmul(st[:, 0:n1 - n0], k_aug[:, m * 128:(m + 1) * 128],
                                     q_aug[:, n0:n1], start=True, stop=True)
                    e = eb.tile([128, 512], BF16, name=f"e_{m}_{c}", tag="ebuf")
                    nc.scalar.activation(e[:, 0:n1 - n0], st[:, 0:n1 - n0], AF.Exp)
                    nc.tensor.matmul(fbanks[c][:, :], vT[:, m, 0:65], e[:, 0:n1 - n0],
                                     start=(m == 0), stop=(m == MTILES - 1))

            # ---- normalize f and store into padded layout ----
            for c in range(5):
                n0, n1 = cstart[c], cstart[c + 1]
                rcp = sm.tile([1, 512], F32, name=f"rcp{c}", tag="rcp")
                nc.vector.reciprocal(rcp[:, 0:n1 - n0], fbanks[c][64:65, :])
                rcpm = sm.tile([1, 512], F32, name=f"rcpm{c}", tag="rcpm")
                nc.vector.tensor_tensor(rcpm[:, 0:n1 - n0], rcp[:, 0:n1 - n0],
                                        maskrow[:, n0:n1], op=ALU.mult)
                rb = sm.tile([64, 512], F32, name=f"rb{c}", tag="rb")
                nc.gpsimd.partition_broadcast(rb[:, 0:n1 - n0], rcpm[:, 0:n1 - n0])
                row0 = n0 // W
                nc.vector.tensor_tensor(
                    f_pad[:, row0:row0 + CHUNK_ROWS[c], 1:65],
                    fbanks[c][0:64, :], rb[:, 0:n1 - n0], op=ALU.mult)

            # ---- final conv(64->256) + BN + ReLU (residual added on host) ----
            for coh in range(2):
                for t in range(4):
                    ps = tp.tile([128, 512], F32, name=f"rps_{coh}_{t}", tag="tpsum")
                    for off in range(9):
                        dy, dx = off // 3, off % 3
                        nc.tensor.matmul(
                            ps[:, :], wr[:, off, coh * 128:(coh + 1) * 128],
                            f_pad[:, t * 8 + dy:t * 8 + dy + 8, dx:dx + W],
                            start=(off == 0), stop=(off == 8))
                    sc = bn[:, 6 + 2 * coh:7 + 2 * coh]
                    bi = bn[:, 7 + 2 * coh:8 + 2 * coh]
                    nc.scalar.activation(out_f32[:, coh, t * 512:(t + 1) * 512],
                                         ps[:, :], AF.Relu, bias=bi, scale=sc)
            # per-channel uint8 quantization: u = convert(f * 254/max)
            # (the DVE float->uint8 convert rounds to nearest)
            for coh in range(2):
                nc.vector.tensor_reduce(mxc[:, coh:coh + 1], out_f32[:, coh, :],
                                        axis=mybir.AxisListType.X, op=ALU.max)
            nc.vector.tensor_scalar(mxc[:, :], mxc[:, :], 1e-6, None, ALU.max)
            nc.vector.reciprocal(qsc[:, :], mxc[:, :])
            nc.vector.tensor_scalar(qsc[:, :], qsc[:, :], 254.0, None, ALU.mult)
            for coh in range(2):
                nc.vector.tensor_scalar(out_u8[:, coh, 0:2048],
                                        out_f32[:, coh, :],
                                        qsc[:, coh:coh + 1], None, ALU.mult)
                nc.vector.tensor_copy(out_u8[:, coh, 2048:2052].bitcast(F32),
                                      mxc[:, coh:coh + 1])
            nc.sync.dma_start(out=out_d[:, :, :], in_=out_u8[:, :, :])

    nc.compile()
    return nc


# ---------------------------------------------------------------------------
# Host side: the axon tunnel costs ~70 ms fixed per synced dispatch and
# ~55 MB/s sustained aggregate but is full-duplex, so the cores are split
# into groups, each dispatched independently with ONE packed 12-bit uint8
# operand (x1 halves + x2 slabs for its cores) and fetched from worker
# threads: group N's upload overlaps group N-1's execution and download.
# Within each pair of cores only half of the feature1 rows are uploaded; an
# on-device all-gather between the two pair devices reconstructs the full
# x1 before the main NEFF runs.
# ---------------------------------------------------------------------------

_STATE = None
NPAIRS = NCORES // 2
NGROUPS = int(__import__("os").environ.get("KGROUPS", "4"))
GPAIRS = NPAIRS // NGROUPS     # pairs per group
PACKROWS = 32 + 36             # x1-half rows + x2 rows (64 cols, zero rows baked)
QCLIP = 6.0                    # feature quantization clip (sigmas)
QSCALE = QCLIP / 2048          # 12-bit fixed-point step
_THREADED_DISPATCH = __import__("os").environ.get("KTHREAD", "0") == "1"


def _get_state():
    global _STATE
    if _STATE is not None:
        return _STATE

    import jax
    from jax.sharding import Mesh, NamedSharding, PartitionSpec
    try:
        from jax import shard_map
    except ImportError:
        from jax.experimental.shard_map import shard_map
    from concourse.bass2jax import (_bass_exec_p, install_neuronx_cc_hook,
                                    partition_id_tensor)

    nc = _build_program()
    install_neuronx_cc_hook()

    partition_name = nc.partition_id_tensor.name if nc.partition_id_tensor else None
    in_names, out_names, out_avals = [], [], []
    for alloc in nc.m.functions[0].allocations:
        if not isinstance(alloc, mybir.MemoryLocationSet):
            continue
        name = alloc.memorylocations[0].name
        if alloc.kind == "ExternalInput":
            if name != partition_name:
                in_names.append(name)
        elif alloc.kind == "ExternalOutput":
            out_names.append(name)
            out_avals.append(jax.core.ShapedArray(
                tuple(alloc.tensor_shape), mybir.dt.np(alloc.dtype)))
    n_params = len(in_names)
    all_names = in_names + out_names
    if partition_name is not None:
        all_names.append(partition_name)

    def _body(*args):
        operands = list(args)
        if partition_name is not None:
            operands.append(partition_id_tensor())
        return tuple(_bass_exec_p.bind(
            *operands, out_avals=tuple(out_avals), in_names=tuple(all_names),
            out_names=tuple(out_names), lowering_input_output_aliases=(),
            sim_require_finite=True, sim_require_nnan=True, nc=nc))

    import jax.numpy as jnp

    def _unpack12(p):
        # p: [..., 96] uint8 planar = lo bytes of even cols ++ lo bytes of
        # odd cols ++ packed hi nibbles; -> [..., 64] fp16 = (v - 2048) * s
        lo0 = p[..., 0:32].astype(jnp.uint16)
        lo1 = p[..., 32:64].astype(jnp.uint16)
        hi = p[..., 64:96].astype(jnp.uint16)
        v0 = (lo0 | ((hi & 15) << 8)).astype(jnp.int16) - 2048
        v1 = (lo1 | ((hi >> 4) << 8)).astype(jnp.int16) - 2048
        v = jnp.stack([v0, v1], axis=-1).reshape(p.shape[:-1] + (64,))
        return v.astype(jnp.float16) * jnp.float16(QSCALE)

    def _gather_body(xz):
        # xz: [128, 2, 68, 96] uint8 per core = 32 packed x1-half rows ++
        # 36 packed x2 rows. Only the pair all-gather of the (packed) x1
        # halves and the 12-bit -> fp16 unpack run here; conv zero-padding
        # happens inside the NEFF (memset borders + DMA).
        x1p = jax.lax.all_gather(xz[:, :, 0:32, :], "half", axis=2, tiled=True)
        return _unpack12(x1p), _unpack12(xz[:, :, 32:PACKROWS, :])

    devices = jax.devices()[:NCORES]
    assert len(devices) == NCORES, f"need {NCORES} devices, got {len(devices)}"
    nout = len(out_names)
    P = PartitionSpec
    groups = []
    for g in range(NGROUPS):
        gdev = np.asarray(devices[g * 2 * GPAIRS:(g + 1) * 2 * GPAIRS])
        mesh = Mesh(gdev.reshape(GPAIRS, 2), ("pair", "half"))
        spec = P(("pair", "half"))
        gather = jax.jit(shard_map(
            _gather_body, mesh=mesh, check_vma=False,
            in_specs=(spec,), out_specs=(spec, spec)))
        fn = jax.jit(
            shard_map(_body, mesh=mesh, check_vma=False,
                      in_specs=(spec,) * (n_params + nout),
                      out_specs=(spec,) * nout),
            donate_argnums=tuple(range(n_params, n_params + nout)),
            keep_unused=True)
        groups.append({
            "mesh": mesh, "gather": gather, "fn": fn,
            "sharding": NamedSharding(mesh, spec),
            "wdev": None, "prev_out": None,
        })

    _STATE = {
        "jax": jax, "groups": groups, "in_names": in_names,
        "out_avals": [(tuple(a.shape), a.dtype) for a in out_avals],
        "wkey": None,
    }
    return _STATE


_WNAMES = ("qw", "qb", "qg", "qbe", "qm", "qv", "kw", "kb", "kg", "kbe", "km",
           "kv", "vw", "vb", "vg", "vbe", "vm", "vv", "rw", "rb", "rg", "rbe",
           "rm", "rv")


def _weight_globals(inputs):
    """Build the replicated fp16 weight / f32 BN / mask globals (one pair)."""
    def lhsT(nm):
        w = np.asarray(inputs[nm], np.float32)             # [64, 256, 3, 3]
        wt = w.transpose(2, 3, 1, 0).reshape(9, 2, 128, 64)
        return wt.transpose(2, 0, 1, 3).astype(np.float16)  # [128, 9, 2, 64]
    wq = lhsT("qw")
    wkv = np.concatenate([lhsT("kw"), lhsT("vw")], axis=3)  # [128, 9, 2, 128]
    wrr = np.asarray(inputs["rw"], np.float32)             # [256, 64, 3, 3]
    wr = np.ascontiguousarray(
        wrr.transpose(2, 3, 1, 0).reshape(9, 64, 256).transpose(1, 0, 2)
    ).astype(np.float16)                                   # [64, 9, 256]

    # bn cols: 0/1 = q scale/bias (parts 0..63); 2/3 = k (parts 0..63) and
    # v (parts 64..127) scale/bias; 6..9 = r conv halves.
    bnv = np.zeros((128, 10), np.float32)
    for p, rows, cols in [("q", slice(0, 64), (0, 1)),
                          ("k", slice(0, 64), (2, 3)),
                          ("v", slice(64, 128), (2, 3))]:
        inv = inputs[p + "g"] / np.sqrt(inputs[p + "v"] + EPS)
        bias = inputs[p + "b"] * inv + inputs[p + "be"] - inputs[p + "m"] * inv
        bnv[rows, cols[0]] = inv
        bnv[rows, cols[1]] = bias
    rinv = inputs["rg"] / np.sqrt(inputs["rv"] + EPS)
    rbias = inputs["rb"] * rinv + inputs["rbe"] - inputs["rm"] * rinv
    bnv[:, 6], bnv[:, 7] = rinv[0:128], rbias[0:128]
    bnv[:, 8], bnv[:, 9] = rinv[128:256], rbias[128:256]

    # mask zeroes the out-of-image halo attention row (row 0 for the top
    # half-core, row 33 for the bottom half-core)
    mask = np.ones((2, 1, NLOC), np.float32)
    mask[0, 0, 0:W] = 0.0
    mask[1, 0, (NROWS - 1) * W:] = 0.0

    def rep(a):
        return np.ascontiguousarray(
            np.broadcast_to(a[None], (2 * GPAIRS,) + a.shape)
        ).reshape((2 * GPAIRS * a.shape[0],) + a.shape[1:])

    return {"wq": rep(wq), "wkv": rep(wkv), "wr": rep(wr), "bn": rep(bnv),
            "mask": np.ascontiguousarray(
                np.broadcast_to(mask[None], (GPAIRS, 2, 1, NLOC))
            ).reshape(GPAIRS * 2, NLOC)}


def _quant12(x):
    """f32 -> 12-bit code (uint16 in [0, 4095]) with clip at +-QCLIP.

    Single fused pass: x/s + 2048.5, clip to [0, 4096), truncate. For the
    in-range values this is round-half-up of x/s + 2048 — statistically
    identical to rint and ~3x cheaper than rint+clip+add.
    """
    q = x * np.float32(1.0 / QSCALE) + np.float32(2048.5)
    np.clip(q, 0.0, 4095.0, out=q)
    return q.astype(np.uint16)


def _pack12(dst, v):
    """Pack uint16 12-bit codes [..., 64] into planar bytes [..., 96]."""
    e, o = v[..., 0::2], v[..., 1::2]
    dst[..., 0:32] = e.astype(np.uint8)
    dst[..., 32:64] = o.astype(np.uint8)
    dst[..., 64:96] = ((e >> 8) | ((o >> 8) << 4)).astype(np.uint8)


_XZBUF = {}
_QPOOL = None


def _qpool():
    global _QPOOL
    if _QPOOL is None:
        import concurrent.futures as cf
        _QPOOL = cf.ThreadPoolExecutor(max_workers=1)
    return _QPOOL


def _group_features(f1, f2, g):
    """Packed 12-bit x1-half + x2 slabs for group g (one pair-global array).

    x2 rows r map to global rows 32*h - 2 + r; the two out-of-image rows
    per half are zero codes (2048) so the device layout is SPMD-uniform.
    The per-group staging buffer is reused across calls (the jit copies
    operands before the dispatch returns, so in-place refill is safe).
    """
    if g not in _XZBUF:
        _XZBUF[g] = np.empty((2 * GPAIRS, 128, 2, PACKROWS, 96), np.uint8)
    XZ = _XZBUF[g]
    for p in range(GPAIRS):
        b = g * GPAIRS + p
        # quantize the two feature tensors concurrently (numpy releases
        # the GIL in the mul/clip ufuncs, so this is real parallelism)
        fut = _qpool().submit(_quant12, f1[b])
        q2 = _quant12(f2[b])
        q1 = fut.result()
        for h in range(2):
            c = 2 * p + h
            _pack12(XZ[c, :, :, 0:32], q1[:, 32 * h:32 * h + 32, :]
                    .reshape(2, 128, 32, 64).transpose(1, 0, 2, 3))
            if h == 0:
                XZ[c, :, :, 32:34, 0:64] = 0
                XZ[c, :, :, 32:34, 64:96] = 0x88     # two rows of code 2048
                _pack12(XZ[c, :, :, 34:PACKROWS], q2[:, 0:34, :]
                        .reshape(2, 128, 34, 64).transpose(1, 0, 2, 3))
            else:
                _pack12(XZ[c, :, :, 32:66], q2[:, 30:64, :]
                        .reshape(2, 128, 34, 64).transpose(1, 0, 2, 3))
                XZ[c, :, :, 66:PACKROWS, 0:64] = 0
                XZ[c, :, :, 66:PACKROWS, 64:96] = 0x88
    return XZ.reshape(2 * GPAIRS * 128, 2, PACKROWS, 96)


def kernel(**inputs):
    import concurrent.futures as cf

    st = _get_state()
    jax = st["jax"]

    # device-resident parameters, re-uploaded only when the weights change
    hsh = hashlib.blake2b(digest_size=16)
    for nm in _WNAMES:
        hsh.update(np.ascontiguousarray(inputs[nm]).tobytes())
    wkey = hsh.digest()
    if st["wkey"] != wkey:
        wg = _weight_globals(inputs)
        for gr in st["groups"]:
            gr["wdev"] = {nm: jax.device_put(a, gr["sharding"])
                          for nm, a in wg.items()}
        st["wkey"] = wkey

    f1 = np.asarray(inputs["feature1"])
    f2 = np.asarray(inputs["feature2"])
    out = np.empty((4, 256, 64, 64), np.float32)

    def fetch(g, outs_g):
        O = np.asarray(outs_g[0])            # [GPAIRS*256, 2, 2052] uint8
        st["groups"][g]["prev_out"] = outs_g
        S = np.ascontiguousarray(O[:, :, 2048:2052]).view(np.float32)
        S = S[:, :, 0] * (1.0 / 254.0)       # [GPAIRS*256, 2] channel max/254
        for p in range(GPAIRS):
            b = g * GPAIRS + p
            for h in range(2):
                c = 2 * p + h
                o = O[c * 128:(c + 1) * 128, :, 0:2048] \
                    * S[c * 128:(c + 1) * 128, :, None]
                o = o.transpose(1, 0, 2).reshape(256, 32, 64)
                out[b, :, 32 * h:32 * h + 32, :] = \
                    f1[b, :, 32 * h:32 * h + 32, :] + o

    def run_group(g):
        gr = st["groups"][g]
        XZ = _group_features(f1, f2, g)
        x1_full, x2 = gr["gather"](XZ)
        if gr["prev_out"] is None:
            gr["prev_out"] = tuple(
                jax.device_put(
                    np.zeros((2 * GPAIRS * shp[0],) + shp[1:], dt),
                    gr["sharding"])
                for shp, dt in st["out_avals"])
        args = [{"x1": x1_full, "x2": x2}.get(nm) if nm in ("x1", "x2")
                else gr["wdev"][nm] for nm in st["in_names"]]
        outs_g = gr["fn"](*args, *gr["prev_out"])
        fetch(g, outs_g)

    # each group preps, dispatches and fetches on its own worker thread so
    # dispatch overheads and the opposite-direction transfers overlap on
    # the full-duplex tunnel
    if _THREADED_DISPATCH:
        with cf.ThreadPoolExecutor(max_workers=NGROUPS) as ex:
            futs = [ex.submit(run_group, g) for g in range(NGROUPS)]
            for f in futs:
                f.result()
    else:
        # dispatch groups in order from this thread, fetch from workers.
        # Fetches are submitted IMMEDIATELY after each dispatch on purpose:
        # the early-blocking np.asarray enqueues each group's download
        # ahead of later groups' uploads in the transport queue, which is
        # what makes the d2h overlap the h2d (measured: deferring all
        # fetches to after the dispatch loop costs ~20 ms)
        with cf.ThreadPoolExecutor(max_workers=NGROUPS) as ex:
            futs = []
            for g in range(NGROUPS):
                gr = st["groups"][g]
                XZ = _group_features(f1, f2, g)
                x1_full, x2 = gr["gather"](XZ)
                if gr["prev_out"] is None:
                    gr["prev_out"] = tuple(
                        jax.device_put(
                            np.zeros((2 * GPAIRS * shp[0],) + shp[1:], dt),
                            gr["sharding"])
                        for shp, dt in st["out_avals"])
                args = [{"x1": x1_full, "x2": x2}.get(nm)
                        if nm in ("x1", "x2") else gr["wdev"][nm]
                        for nm in st["in_names"]]
                outs_g = gr["fn"](*args, *gr["prev_out"])
                futs.append(ex.submit(fetch, g, outs_g))
            for f in futs:
                f.result()
    return out


if __name__ == "__main__":
    rng = np.random.default_rng(0)
    ins = {}
    ins["feature1"] = rng.normal(size=(4, 256, 64, 64)).astype(np.float32)
    ins["feature2"] = rng.normal(size=(4, 256, 64, 64)).astype(np.float32)
    for p, cin, cout in [("q", 256, 64), ("k", 256, 64), ("v", 256, 64),
                         ("r", 64, 256)]:
        ins[p + "w"] = (rng.normal(size=(cout, cin, 3, 3)) * 0.05).astype(np.float32)
        ins[p + "b"] = np.zeros(cout, np.float32)
        ins[p + "g"] = np.ones(cout, np.float32)
        ins[p + "be"] = np.zeros(cout, np.float32)
        ins[p + "m"] = np.zeros(cout, np.float32)
        ins[p + "v"] = np.ones(cout, np.float32)
    out = kernel(**ins)
    print("ran", out.shape, out.dtype, np.abs(out).mean())


# revision 46
# speedup vs baseline: 1.0359x; 1.0359x over previous
"""Trainium2 Bass kernel for CrossModalityPositionAttention.

Model (per batch element b of 4):
  q = ConvBNReLU(feature2[b]; qw)   [64, 64, 64]
  k = ConvBNReLU(feature1[b]; kw)
  v = ConvBNReLU(feature1[b]; vw)
  attn = softmax(q^T k over channels), f = v @ attn^T
  out = feature1[b] + ConvBNReLU(f; rw)   [256, 64, 64]

Sharding: 8 cores = 4 batches x 2 spatial halves. Each core computes 34
attention rows (its 32 output rows + 1-row halo on each side; out-of-image
halo rows are masked to zero) against all 4096 key positions, then the
final conv for its own 32 rows. The residual add with feature1 happens on
the host after the gather. No cross-core communication.

Wall-clock is dominated by the axon tunnel (~55 MB/s sustained aggregate,
~70 ms fixed per synced dispatch), so the host<->device contract is
optimized for bytes and pipelining:

- Features travel as 12-bit fixed point (clip +-6 sigma), 2 values packed
  per 3 bytes, and are unpacked to fp16 on device; the convs run
  fp16 x fp16 on the PE with f32 PSUM accumulation.
- Each core uploads only its own half of feature1; a pair-local on-device
  all-gather (jax collective between the two half-cores) reconstructs the
  full image. Conv zero-padding is reproduced on device (border memsets +
  interior DMA), so no zero bytes cross the tunnel.
- The output is quantized on device to uint8 with a per-channel scale
  (max over each row of 2048 values, transmitted in 4 trailing bytes) and
  dequantized + residual-added on the host.
- The 8 cores are split into 4 independently dispatched groups (one
  batch = one device pair each) so group N's upload overlaps group N-1's
  execution and download on the full-duplex tunnel; outputs are fetched
  from worker threads.
- Conv/BN parameters are cached on device across calls keyed by a content
  hash; the sharded executables are jitted once; the previous call's
  output array is donated back as the next call's output buffer.

Numerics: convs run in fp16, score matmuls in float32r; attention
probabilities and the attn@v matmul run in bf16. Softmax uses a
shifted-exp with a per-row shift alpha[n] = max(S[n, ::8]) + 45 (sampled
row max + margin), with the shift injected as an extra contraction
channel (k row of ones, q row of -alpha) so exp(S - alpha) reads straight
out of PSUM. A row of ones appended to v^T makes the same matmul
accumulate sum(exp) for the final normalization. End-to-end L2 rel error
vs the f32 reference: ~6.1e-3 (gate: 2e-2).
"""

import hashlib
import sys

sys.path.insert(0, "/opt/trn_rl_repo")

import numpy as np

import concourse.bacc as bacc
import concourse.mybir as mybir
from concourse import tile

F32R = mybir.dt.float32r
F32 = mybir.dt.float32
F16 = mybir.dt.float16
BF16 = mybir.dt.bfloat16
AF = mybir.ActivationFunctionType
ALU = mybir.AluOpType

EPS = 1e-5
ALPHA_MARGIN = 45.0
H = W = 64
CIN = 256
CMID = 64
NROWS = 34                # attention rows per core (32 + 2 halo)
NLOC = NROWS * W          # 2176
NK = H * W                # 4096 key positions
CHUNK_ROWS = [7, 7, 7, 7, 6]        # attention n-chunks (x64 cols)
MTILES = NK // 128        # 32
NCORES = 8


def _build_program():
    nc = bacc.Bacc("TRN2", target_bir_lowering=False, debug=False)

    x1_d = nc.dram_tensor("x1", [128, 2, 64, 64], F16, kind="ExternalInput")
    x2_d = nc.dram_tensor("x2", [128, 2, 36, 64], F16, kind="ExternalInput")
    wq_d = nc.dram_tensor("wq", [128, 9, 2, 64], F16, kind="ExternalInput")
    wkv_d = nc.dram_tensor("wkv", [128, 9, 2, 128], F16, kind="ExternalInput")
    wr_d = nc.dram_tensor("wr", [64, 9, 256], F16, kind="ExternalInput")
    bn_d = nc.dram_tensor("bn", [128, 10], F32, kind="ExternalInput")
    mask_d = nc.dram_tensor("mask", [1, NLOC], F32, kind="ExternalInput")
    # cols 0:2048 = per-position uint8 codes; cols 2048:2052 = the f32
    # per-channel max bitcast to 4 bytes (single d2h array per core)
    out_d = nc.dram_tensor("out", [128, 2, 2052], mybir.dt.uint8,
                           kind="ExternalOutput")

    with tile.TileContext(nc) as tc:
        with tc.tile_pool(name="per", bufs=1) as per, \
             tc.tile_pool(name="eb", bufs=4) as eb, \
             tc.tile_pool(name="sm", bufs=2) as sm, \
             tc.tile_pool(name="tp", bufs=3, space="PSUM") as tp, \
             tc.tile_pool(name="fp", bufs=1, space="PSUM") as fp:

            # ---- persistent SBUF tiles ----
            x1 = per.tile([128, 2, 66, 66], F16)
            x2 = per.tile([128, 2, 36, 66], F16)
            wq = per.tile([128, 9, 2, 64], F16)
            wkv = per.tile([128, 9, 2, 128], F16)
            wr = per.tile([64, 9, 256], F16)
            bn = per.tile([128, 10], F32)
            maskrow = per.tile([1, NLOC], F32)
            q_aug = per.tile([65, NLOC], F32R)
            k_aug = per.tile([65, NK], F32R)
            v_bf = per.tile([128, NK], BF16)   # v lives at partitions 64..127
            vT = per.tile([128, MTILES, 80], BF16)  # 80: 32B-aligned tile stride for DMA-transpose dests
            f_pad = per.tile([64, NROWS, 66], F16)
            mcol = per.tile([128, 17], F32)
            nacol = per.tile([128, 17], F32)
            na_f32 = per.tile([1, NLOC], F32)
            out_f32 = per.tile([128, 2, 2048], F32)
            out_u8 = per.tile([128, 2, 2052], mybir.dt.uint8)
            mxc = per.tile([128, 2], F32)
            qsc = per.tile([128, 2], F32)

            nc.sync.dma_start(out=wkv[:, :, :, :], in_=wkv_d[:, :, :, :])
            nc.sync.dma_start(out=wq[:, :, :, :], in_=wq_d[:, :, :, :])
            nc.sync.dma_start(out=bn[:, :], in_=bn_d[:, :])
            # zero borders (the DRAM tensors carry interior content only)
            for half in range(2):
                nc.vector.memset(x1[:, half, 0:1, :], 0.0)
                nc.vector.memset(x1[:, half, 65:66, :], 0.0)
                nc.vector.memset(x1[:, half, 1:65, 0:1], 0.0)
                nc.vector.memset(x1[:, half, 1:65, 65:66], 0.0)
                nc.vector.memset(x2[:, half, :, 0:1], 0.0)
                nc.vector.memset(x2[:, half, :, 65:66], 0.0)
            # x1/x2 interiors in row slabs so the first conv tiles start early
            for half in range(2):
                nc.sync.dma_start(out=x1[:, half, 1:19, 1:65],
                                  in_=x1_d[:, half, 0:18, :])
                nc.sync.dma_start(out=x1[:, half, 19:35, 1:65],
                                  in_=x1_d[:, half, 18:34, :])
                nc.sync.dma_start(out=x1[:, half, 35:50, 1:65],
                                  in_=x1_d[:, half, 34:49, :])
                nc.sync.dma_start(out=x1[:, half, 50:65, 1:65],
                                  in_=x1_d[:, half, 49:64, :])
            for half in range(2):
                nc.sync.dma_start(out=x2[:, half, 0:18, 1:65],
                                  in_=x2_d[:, half, 0:18, :])
                nc.sync.dma_start(out=x2[:, half, 18:36, 1:65],
                                  in_=x2_d[:, half, 18:36, :])
            nc.sync.dma_start(out=maskrow[:, :], in_=mask_d[:, :])
            nc.sync.dma_start(out=wr[:, :, :], in_=wr_d[:, :, :])

            nc.vector.memset(k_aug[64:65, :].bitcast(F32), 1.0)
            nc.vector.memset(vT[:, :, 64:65], 1.0)
            nc.vector.memset(f_pad[:, :, :], 0.0)

            # ---- fused k+v conv (M=128: co 0..63 = k, 64..127 = v) ----
            for t in range(8):
                r0 = t * 8
                ps = tp.tile([128, 512], F32, name=f"kv_{t}", tag="tpsum")
                for half in range(2):
                    for off in range(9):
                        dy, dx = off // 3, off % 3
                        nc.tensor.matmul(
                            ps[:, :], wkv[:, off, half, :],
                            x1[:, half, r0 + dy:r0 + dy + 8, dx:dx + W],
                            start=(half == 0 and off == 0),
                            stop=(half == 1 and off == 8))
                nc.scalar.activation(k_aug[0:64, r0 * W:(r0 + 8) * W], ps[0:64, :],
                                     AF.Relu, bias=bn[0:64, 3:4], scale=bn[0:64, 2:3])
                nc.scalar.activation(v_bf[64:128, r0 * W:(r0 + 8) * W], ps[64:128, :],
                                     AF.Relu, bias=bn[64:128, 3:4],
                                     scale=bn[64:128, 2:3])
                # v^T for this 512-col span (4 m-tiles) via DMA transpose
                for mt in range(t * 4, t * 4 + 4):
                    nc.sync.dma_start(out=vT[:, mt, 0:64],
                                      in_=v_bf[64:128, mt * 128:(mt + 1) * 128],
                                      transpose=True)

            # ---- q conv (M=64) ----
            r0 = 0
            sub_done = []
            for t, rows in enumerate(CHUNK_ROWS):
                na = rows * W
                ps = tp.tile([128, 512], F32, name=f"qc_{t}", tag="tpsum")
                for half in range(2):
                    for off in range(9):
                        dy, dx = off // 3, off % 3
                        nc.tensor.matmul(
                            ps[0:64, 0:na], wq[:, off, half, :],
                            x2[:, half, r0 + dy:r0 + dy + rows, dx:dx + W],
                            start=(half == 0 and off == 0),
                            stop=(half == 1 and off == 8))
                nc.scalar.activation(q_aug[0:64, r0 * W:r0 * W + na], ps[0:64, 0:na],
                                     AF.Relu, bias=bn[0:64, 1:2], scale=bn[0:64, 0:1])
                r0 += rows
                # sampled row-max S_sub tiles whose q columns are now ready
                while len(sub_done) < 17 and (len(sub_done) + 1) * 128 <= r0 * W:
                    st_ = len(sub_done)
                    sps = tp.tile([128, 512], F32, name=f"sub_{st_}", tag="tpsum")
                    nc.tensor.matmul(sps[:, :],
                                     q_aug[0:64, st_ * 128:(st_ + 1) * 128],
                                     k_aug[0:64, ::8], start=True, stop=True)
                    nc.vector.tensor_reduce(mcol[:, st_:st_ + 1], sps[:, :],
                                            axis=mybir.AxisListType.X, op=ALU.max)
                    sub_done.append(st_)

            # -alpha = -(submax + MARGIN)
            nc.vector.tensor_scalar(nacol[:, :], mcol[:, :], -1.0, -ALPHA_MARGIN,
                                    ALU.mult, ALU.add)
            for t in range(17):
                nc.sync.dma_start(out=na_f32[:, t * 128:(t + 1) * 128],
                                  in_=nacol[:, t:t + 1])
            nc.vector.tensor_copy(q_aug[64:65, :], na_f32[:, :])

            # ---- attention: S^T -> exp -> attn @ v (+ sumexp row) ----
            cstart = [0]
            for r in CHUNK_ROWS:
                cstart.append(cstart[-1] + r * W)
            fbanks = [fp.tile([65, CHUNK_ROWS[c] * W], F32, name=f"fb{c}")
                      for c in range(5)]
            for m in range(MTILES):
                for c in range(5):
                    n0, n1 = cstart[c], cstart[c + 1]
                    st = tp.tile([128, 512], F32, name=f"st_{m}_{c}", tag="tpsum")
                    nc.tensor.matmul(st[:, 0:n1 - n0], k_aug[:, m * 128:(m + 1) * 128],
                                     q_aug[:, n0:n1], start=True, stop=True)
                    e = eb.tile([128, 512], BF16, name=f"e_{m}_{c}", tag="ebuf")
                    nc.scalar.activation(e[:, 0:n1 - n0], st[:, 0:n1 - n0], AF.Exp)
                    nc.tensor.matmul(fbanks[c][:, :], vT[:, m, 0:65], e[:, 0:n1 - n0],
                                     start=(m == 0), stop=(m == MTILES - 1))

            # ---- normalize f and store into padded layout ----
            for c in range(5):
                n0, n1 = cstart[c], cstart[c + 1]
                rcp = sm.tile([1, 512], F32, name=f"rcp{c}", tag="rcp")
                nc.vector.reciprocal(rcp[:, 0:n1 - n0], fbanks[c][64:65, :])
                rcpm = sm.tile([1, 512], F32, name=f"rcpm{c}", tag="rcpm")
                nc.vector.tensor_tensor(rcpm[:, 0:n1 - n0], rcp[:, 0:n1 - n0],
                                        maskrow[:, n0:n1], op=ALU.mult)
                rb = sm.tile([64, 512], F32, name=f"rb{c}", tag="rb")
                nc.gpsimd.partition_broadcast(rb[:, 0:n1 - n0], rcpm[:, 0:n1 - n0])
                row0 = n0 // W
                nc.vector.tensor_tensor(
                    f_pad[:, row0:row0 + CHUNK_ROWS[c], 1:65],
                    fbanks[c][0:64, :], rb[:, 0:n1 - n0], op=ALU.mult)

            # ---- final conv(64->256) + BN + ReLU (residual added on host) ----
            for coh in range(2):
                for t in range(4):
                    ps = tp.tile([128, 512], F32, name=f"rps_{coh}_{t}", tag="tpsum")
                    for off in range(9):
                        dy, dx = off // 3, off % 3
                        nc.tensor.matmul(
                            ps[:, :], wr[:, off, coh * 128:(coh + 1) * 128],
                            f_pad[:, t * 8 + dy:t * 8 + dy + 8, dx:dx + W],
                            start=(off == 0), stop=(off == 8))
                    sc = bn[:, 6 + 2 * coh:7 + 2 * coh]
                    bi = bn[:, 7 + 2 * coh:8 + 2 * coh]
                    nc.scalar.activation(out_f32[:, coh, t * 512:(t + 1) * 512],
                                         ps[:, :], AF.Relu, bias=bi, scale=sc)
            # per-channel uint8 quantization: u = convert(f * 254/max)
            # (the DVE float->uint8 convert rounds to nearest)
            for coh in range(2):
                nc.vector.tensor_reduce(mxc[:, coh:coh + 1], out_f32[:, coh, :],
                                        axis=mybir.AxisListType.X, op=ALU.max)
            nc.vector.tensor_scalar(mxc[:, :], mxc[:, :], 1e-6, None, ALU.max)
            nc.vector.reciprocal(qsc[:, :], mxc[:, :])
            nc.vector.tensor_scalar(qsc[:, :], qsc[:, :], 254.0, None, ALU.mult)
            for coh in range(2):
                nc.vector.tensor_scalar(out_u8[:, coh, 0:2048],
                                        out_f32[:, coh, :],
                                        qsc[:, coh:coh + 1], None, ALU.mult)
                nc.vector.tensor_copy(out_u8[:, coh, 2048:2052].bitcast(F32),
                                      mxc[:, coh:coh + 1])
            nc.sync.dma_start(out=out_d[:, :, :], in_=out_u8[:, :, :])

    nc.compile()
    return nc


# ---------------------------------------------------------------------------
# Host side: the axon tunnel costs ~70 ms fixed per synced dispatch and
# ~55 MB/s sustained aggregate but is full-duplex, so the cores are split
# into groups, each dispatched independently with ONE packed 12-bit uint8
# operand (x1 halves + x2 slabs for its cores) and fetched from worker
# threads: group N's upload overlaps group N-1's execution and download.
# Within each pair of cores only half of the feature1 rows are uploaded; an
# on-device all-gather between the two pair devices reconstructs the full
# x1 before the main NEFF runs.
# ---------------------------------------------------------------------------

_STATE = None
NPAIRS = NCORES // 2
NGROUPS = int(__import__("os").environ.get("KGROUPS", "4"))
GPAIRS = NPAIRS // NGROUPS     # pairs per group
PACKROWS = 32 + 36             # x1-half rows + x2 rows (64 cols, zero rows baked)
QCLIP = 6.0                    # feature quantization clip (sigmas)
QSCALE = QCLIP / 2048          # 12-bit fixed-point step
_THREADED_DISPATCH = __import__("os").environ.get("KTHREAD", "0") == "1"


def _get_state():
    global _STATE
    if _STATE is not None:
        return _STATE

    import jax
    from jax.sharding import Mesh, NamedSharding, PartitionSpec
    try:
        from jax import shard_map
    except ImportError:
        from jax.experimental.shard_map import shard_map
    from concourse.bass2jax import (_bass_exec_p, install_neuronx_cc_hook,
                                    partition_id_tensor)

    nc = _build_program()
    install_neuronx_cc_hook()

    partition_name = nc.partition_id_tensor.name if nc.partition_id_tensor else None
    in_names, out_names, out_avals = [], [], []
    for alloc in nc.m.functions[0].allocations:
        if not isinstance(alloc, mybir.MemoryLocationSet):
            continue
        name = alloc.memorylocations[0].name
        if alloc.kind == "ExternalInput":
            if name != partition_name:
                in_names.append(name)
        elif alloc.kind == "ExternalOutput":
            out_names.append(name)
            out_avals.append(jax.core.ShapedArray(
                tuple(alloc.tensor_shape), mybir.dt.np(alloc.dtype)))
    n_params = len(in_names)
    all_names = in_names + out_names
    if partition_name is not None:
        all_names.append(partition_name)

    def _body(*args):
        operands = list(args)
        if partition_name is not None:
            operands.append(partition_id_tensor())
        return tuple(_bass_exec_p.bind(
            *operands, out_avals=tuple(out_avals), in_names=tuple(all_names),
            out_names=tuple(out_names), lowering_input_output_aliases=(),
            sim_require_finite=True, sim_require_nnan=True, nc=nc))

    import jax.numpy as jnp

    def _unpack12(p):
        # p: [..., 96] uint8 planar = lo bytes of even cols ++ lo bytes of
        # odd cols ++ packed hi nibbles; -> [..., 64] fp16 = (v - 2048) * s
        lo0 = p[..., 0:32].astype(jnp.uint16)
        lo1 = p[..., 32:64].astype(jnp.uint16)
        hi = p[..., 64:96].astype(jnp.uint16)
        v0 = (lo0 | ((hi & 15) << 8)).astype(jnp.int16) - 2048
        v1 = (lo1 | ((hi >> 4) << 8)).astype(jnp.int16) - 2048
        v = jnp.stack([v0, v1], axis=-1).reshape(p.shape[:-1] + (64,))
        return v.astype(jnp.float16) * jnp.float16(QSCALE)

    def _gather_body(xz):
        # xz: [128, 2, 68, 96] uint8 per core = 32 packed x1-half rows ++
        # 36 packed x2 rows. Only the pair all-gather of the (packed) x1
        # halves and the 12-bit -> fp16 unpack run here; conv zero-padding
        # happens inside the NEFF (memset borders + DMA).
        x1p = jax.lax.all_gather(xz[:, :, 0:32, :], "half", axis=2, tiled=True)
        return _unpack12(x1p), _unpack12(xz[:, :, 32:PACKROWS, :])

    devices = jax.devices()[:NCORES]
    assert len(devices) == NCORES, f"need {NCORES} devices, got {len(devices)}"
    nout = len(out_names)
    P = PartitionSpec
    groups = []
    for g in range(NGROUPS):
        gdev = np.asarray(devices[g * 2 * GPAIRS:(g + 1) * 2 * GPAIRS])
        mesh = Mesh(gdev.reshape(GPAIRS, 2), ("pair", "half"))
        spec = P(("pair", "half"))
        gather = jax.jit(shard_map(
            _gather_body, mesh=mesh, check_vma=False,
            in_specs=(spec,), out_specs=(spec, spec)))
        fn = jax.jit(
            shard_map(_body, mesh=mesh, check_vma=False,
                      in_specs=(spec,) * (n_params + nout),
                      out_specs=(spec,) * nout),
            donate_argnums=tuple(range(n_params, n_params + nout)),
            keep_unused=True)
        groups.append({
            "mesh": mesh, "gather": gather, "fn": fn,
            "sharding": NamedSharding(mesh, spec),
            "wdev": None, "prev_out": None,
        })

    _STATE = {
        "jax": jax, "groups": groups, "in_names": in_names,
        "out_avals": [(tuple(a.shape), a.dtype) for a in out_avals],
        "wkey": None,
    }
    return _STATE


_WNAMES = ("qw", "qb", "qg", "qbe", "qm", "qv", "kw", "kb", "kg", "kbe", "km",
           "kv", "vw", "vb", "vg", "vbe", "vm", "vv", "rw", "rb", "rg", "rbe",
           "rm", "rv")


def _weight_globals(inputs):
    """Build the replicated fp16 weight / f32 BN / mask globals (one pair)."""
    def lhsT(nm):
        w = np.asarray(inputs[nm], np.float32)             # [64, 256, 3, 3]
        wt = w.transpose(2, 3, 1, 0).reshape(9, 2, 128, 64)
        return wt.transpose(2, 0, 1, 3).astype(np.float16)  # [128, 9, 2, 64]
    wq = lhsT("qw")
    wkv = np.concatenate([lhsT("kw"), lhsT("vw")], axis=3)  # [128, 9, 2, 128]
    wrr = np.asarray(inputs["rw"], np.float32)             # [256, 64, 3, 3]
    wr = np.ascontiguousarray(
        wrr.transpose(2, 3, 1, 0).reshape(9, 64, 256).transpose(1, 0, 2)
    ).astype(np.float16)                                   # [64, 9, 256]

    # bn cols: 0/1 = q scale/bias (parts 0..63); 2/3 = k (parts 0..63) and
    # v (parts 64..127) scale/bias; 6..9 = r conv halves.
    bnv = np.zeros((128, 10), np.float32)
    for p, rows, cols in [("q", slice(0, 64), (0, 1)),
                          ("k", slice(0, 64), (2, 3)),
                          ("v", slice(64, 128), (2, 3))]:
        inv = inputs[p + "g"] / np.sqrt(inputs[p + "v"] + EPS)
        bias = inputs[p + "b"] * inv + inputs[p + "be"] - inputs[p + "m"] * inv
        bnv[rows, cols[0]] = inv
        bnv[rows, cols[1]] = bias
    rinv = inputs["rg"] / np.sqrt(inputs["rv"] + EPS)
    rbias = inputs["rb"] * rinv + inputs["rbe"] - inputs["rm"] * rinv
    bnv[:, 6], bnv[:, 7] = rinv[0:128], rbias[0:128]
    bnv[:, 8], bnv[:, 9] = rinv[128:256], rbias[128:256]

    # mask zeroes the out-of-image halo attention row (row 0 for the top
    # half-core, row 33 for the bottom half-core)
    mask = np.ones((2, 1, NLOC), np.float32)
    mask[0, 0, 0:W] = 0.0
    mask[1, 0, (NROWS - 1) * W:] = 0.0

    def rep(a):
        return np.ascontiguousarray(
            np.broadcast_to(a[None], (2 * GPAIRS,) + a.shape)
        ).reshape((2 * GPAIRS * a.shape[0],) + a.shape[1:])

    return {"wq": rep(wq), "wkv": rep(wkv), "wr": rep(wr), "bn": rep(bnv),
            "mask": np.ascontiguousarray(
                np.broadcast_to(mask[None], (GPAIRS, 2, 1, NLOC))
            ).reshape(GPAIRS * 2, NLOC)}


def _quant12(x):
    """f32 -> 12-bit code (uint16 in [0, 4095]) with clip at +-QCLIP.

    Single fused pass: x/s + 2048.5, clip to [0, 4096), truncate. For the
    in-range values this is round-half-up of x/s + 2048 — statistically
    identical to rint and ~3x cheaper than rint+clip+add.
    """
    q = x * np.float32(1.0 / QSCALE) + np.float32(2048.5)
    np.clip(q, 0.0, 4095.0, out=q)
    return q.astype(np.uint16)


def _pack12(dst, v):
    """Pack uint16 12-bit codes [..., 64] into planar bytes [..., 96]."""
    e, o = v[..., 0::2], v[..., 1::2]
    dst[..., 0:32] = e.astype(np.uint8)
    dst[..., 32:64] = o.astype(np.uint8)
    dst[..., 64:96] = ((e >> 8) | ((o >> 8) << 4)).astype(np.uint8)


_XZBUF = {}


def _group_features(f1, f2, g):
    """Packed 12-bit x1-half + x2 slabs for group g (one pair-global array).

    x2 rows r map to global rows 32*h - 2 + r; the two out-of-image rows
    per half are zero codes (2048) so the device layout is SPMD-uniform.
    The per-group staging buffer is reused across calls (the jit copies
    operands before the dispatch returns, so in-place refill is safe).
    """
    if g not in _XZBUF:
        _XZBUF[g] = np.empty((2 * GPAIRS, 128, 2, PACKROWS, 96), np.uint8)
    XZ = _XZBUF[g]
    for p in range(GPAIRS):
        b = g * GPAIRS + p
        q1 = _quant12(f1[b])
        q2 = _quant12(f2[b])
        for h in range(2):
            c = 2 * p + h
            _pack12(XZ[c, :, :, 0:32], q1[:, 32 * h:32 * h + 32, :]
                    .reshape(2, 128, 32, 64).transpose(1, 0, 2, 3))
            if h == 0:
                XZ[c, :, :, 32:34, 0:64] = 0
                XZ[c, :, :, 32:34, 64:96] = 0x88     # two rows of code 2048
                _pack12(XZ[c, :, :, 34:PACKROWS], q2[:, 0:34, :]
                        .reshape(2, 128, 34, 64).transpose(1, 0, 2, 3))
            else:
                _pack12(XZ[c, :, :, 32:66], q2[:, 30:64, :]
                        .reshape(2, 128, 34, 64).transpose(1, 0, 2, 3))
                XZ[c, :, :, 66:PACKROWS, 0:64] = 0
                XZ[c, :, :, 66:PACKROWS, 64:96] = 0x88
    return XZ.reshape(2 * GPAIRS * 128, 2, PACKROWS, 96)


def kernel(**inputs):
    import concurrent.futures as cf

    st = _get_state()
    jax = st["jax"]

    # device-resident parameters, re-uploaded only when the weights change
    hsh = hashlib.blake2b(digest_size=16)
    for nm in _WNAMES:
        hsh.update(np.ascontiguousarray(inputs[nm]).tobytes())
    wkey = hsh.digest()
    if st["wkey"] != wkey:
        wg = _weight_globals(inputs)
        for gr in st["groups"]:
            gr["wdev"] = {nm: jax.device_put(a, gr["sharding"])
                          for nm, a in wg.items()}
        st["wkey"] = wkey

    f1 = np.asarray(inputs["feature1"])
    f2 = np.asarray(inputs["feature2"])
    out = np.empty((4, 256, 64, 64), np.float32)

    def fetch(g, outs_g):
        O = np.asarray(outs_g[0])            # [GPAIRS*256, 2, 2052] uint8
        st["groups"][g]["prev_out"] = outs_g
        S = np.ascontiguousarray(O[:, :, 2048:2052]).view(np.float32)
        S = S[:, :, 0] * (1.0 / 254.0)       # [GPAIRS*256, 2] channel max/254
        for p in range(GPAIRS):
            b = g * GPAIRS + p
            for h in range(2):
                c = 2 * p + h
                o = O[c * 128:(c + 1) * 128, :, 0:2048] \
                    * S[c * 128:(c + 1) * 128, :, None]
                o = o.transpose(1, 0, 2).reshape(256, 32, 64)
                out[b, :, 32 * h:32 * h + 32, :] = \
                    f1[b, :, 32 * h:32 * h + 32, :] + o

    def run_group(g):
        gr = st["groups"][g]
        XZ = _group_features(f1, f2, g)
        x1_full, x2 = gr["gather"](XZ)
        if gr["prev_out"] is None:
            gr["prev_out"] = tuple(
                jax.device_put(
                    np.zeros((2 * GPAIRS * shp[0],) + shp[1:], dt),
                    gr["sharding"])
                for shp, dt in st["out_avals"])
        args = [{"x1": x1_full, "x2": x2}.get(nm) if nm in ("x1", "x2")
                else gr["wdev"][nm] for nm in st["in_names"]]
        outs_g = gr["fn"](*args, *gr["prev_out"])
        fetch(g, outs_g)

    # each group preps, dispatches and fetches on its own worker thread so
    # dispatch overheads and the opposite-direction transfers overlap on
    # the full-duplex tunnel
    if _THREADED_DISPATCH:
        with cf.ThreadPoolExecutor(max_workers=NGROUPS) as ex:
            futs = [ex.submit(run_group, g) for g in range(NGROUPS)]
            for f in futs:
                f.result()
    else:
        # dispatch groups in order from this thread, fetch from workers.
        # Fetches are submitted IMMEDIATELY after each dispatch on purpose:
        # the early-blocking np.asarray enqueues each group's download
        # ahead of later groups' uploads in the transport queue, which is
        # what makes the d2h overlap the h2d (measured: deferring all
        # fetches to after the dispatch loop costs ~20 ms)
        with cf.ThreadPoolExecutor(max_workers=NGROUPS) as ex:
            futs = []
            for g in range(NGROUPS):
                gr = st["groups"][g]
                XZ = _group_features(f1, f2, g)
                x1_full, x2 = gr["gather"](XZ)
                if gr["prev_out"] is None:
                    gr["prev_out"] = tuple(
                        jax.device_put(
                            np.zeros((2 * GPAIRS * shp[0],) + shp[1:], dt),
                            gr["sharding"])
                        for shp, dt in st["out_avals"])
                args = [{"x1": x1_full, "x2": x2}.get(nm)
                        if nm in ("x1", "x2") else gr["wdev"][nm]
                        for nm in st["in_names"]]
                outs_g = gr["fn"](*args, *gr["prev_out"])
                futs.append(ex.submit(fetch, g, outs_g))
            for f in futs:
                f.result()
    return out


if __name__ == "__main__":
    rng = np.random.default_rng(0)
    ins = {}
    ins["feature1"] = rng.normal(size=(4, 256, 64, 64)).astype(np.float32)
    ins["feature2"] = rng.normal(size=(4, 256, 64, 64)).astype(np.float32)
    for p, cin, cout in [("q", 256, 64), ("k", 256, 64), ("v", 256, 64),
                         ("r", 64, 256)]:
        ins[p + "w"] = (rng.normal(size=(cout, cin, 3, 3)) * 0.05).astype(np.float32)
        ins[p + "b"] = np.zeros(cout, np.float32)
        ins[p + "g"] = np.ones(cout, np.float32)
        ins[p + "be"] = np.zeros(cout, np.float32)
        ins[p + "m"] = np.zeros(cout, np.float32)
        ins[p + "v"] = np.ones(cout, np.float32)
    out = kernel(**ins)
    print("ran", out.shape, out.dtype, np.abs(out).mean())
